# revision 7
# baseline (speedup 1.0000x reference)
"""Banded causal self-attention (band width 64) on 8 trn2 NeuronCores.

Sequence-parallel sharding: core c handles batch c//4, query block c%4
(512 queries of T=2048), recomputing a 64-token k/v halo locally so no
collectives are needed. The host casts inputs to bf16, transposes x per
core, and supplies the additive band mask; the device kernel fuses
qkv-projection -> banded attention -> output projection.

Layouts on device (per core):
  xt   [C, 576]      x chunk transposed (64-token halo + 512 owned tokens)
  qk^T [2048, 576]   q/k feature-major (16 slabs of 128; head h -> slab h//2
                     (+8 for k), rows (h%2)*64 .. +64)
  v    [576, 1024]   token-major (5 row tiles of 128/64)
  y^T  [1024, 512]   attention output feature-major
  out  [512, 1024]   final tokens x C
Softmax skips max-subtraction (scores are O(1) for this problem); the
band mask is additive (-1e9) and exp applies the 1/sqrt(D) scale.
"""

import numpy as np
import ml_dtypes

import concourse.mybir as mybir
import concourse.tile as tile
from concourse import bacc
from concourse import bass_utils
from concourse.masks import make_identity

B, T, C, H, D = 2, 2048, 1024, 16, 64
W = 64            # band width: key j visible to query i iff i-64 <= j <= i
N_CORES = 8
QL = 512          # queries per core
HT = QL + W       # tokens incl. halo
P = 128
NQT = QL // P     # query tiles per core
KW = P + W        # key window per query tile
KC = C // P       # contraction chunks
NFT = 2 * C // P  # q|k feature slabs
NEG = -1e9

bf16 = mybir.dt.bfloat16
f32 = mybir.dt.float32
Act = mybir.ActivationFunctionType
X = mybir.AxisListType.X

_CACHE = {}


def _emit(tc, xt, wqk, wv, wp, bqk, bvr, bpr, mask, out):
    nc = tc.nc
    with (
        tc.tile_pool(name="const", bufs=1) as const,
        tc.tile_pool(name="wqkp", bufs=3) as wqkp,
        tc.tile_pool(name="attn", bufs=4) as at,
        tc.tile_pool(name="ot", bufs=3) as ot,
        tc.tile_pool(name="psA", bufs=2, space="PSUM") as psA,
        tc.tile_pool(name="psS", bufs=2, space="PSUM") as psSp,
        tc.tile_pool(name="psT", bufs=2, space="PSUM") as psTp,
        tc.tile_pool(name="psY", bufs=2, space="PSUM") as psYp,
    ):
        # ---- persistent tiles ----
        xt_sb = const.tile([P, KC, HT], bf16)
        nc.sync.dma_start(xt_sb[:], xt.rearrange("(kc p) t -> p kc t", p=P))
        wv_sb = const.tile([P, KC, C], bf16)
        nc.sync.dma_start(wv_sb[:], wv.rearrange("(kc p) n -> p kc n", p=P))
        wp_sb = const.tile([P, KC, C], bf16)
        nc.sync.dma_start(wp_sb[:], wp.rearrange("(kc p) n -> p kc n", p=P))
        mask_sb = const.tile([P, NQT, KW], f32)
        nc.sync.dma_start(mask_sb[:], mask.rearrange("t p k -> p t k"))
        bqk_sb = const.tile([P, NFT], f32)
        nc.sync.dma_start(bqk_sb[:], bqk.rearrange("(ft p) -> p ft", p=P))
        bvr_sb = const.tile([P, C], f32)
        nc.sync.dma_start(bvr_sb[:], bvr[:])
        bpr_sb = const.tile([P, C], f32)
        nc.sync.dma_start(bpr_sb[:], bpr[:])
        ident = const.tile([P, P], bf16)
        make_identity(nc, ident[:])

        qkT_sb = const.tile([P, NFT, HT], bf16)   # q^T | k^T feature-major
        v_sb = const.tile([P, NQT + 1, C], bf16)    # v token-major
        yT_sb = const.tile([P, KC, QL], bf16)       # attn out feature-major

        # ---- phase 1a: qk^T = Wqk^T @ x^T  (feature-major) ----
        for ft in range(NFT):
            wt = wqkp.tile([P, KC, P], bf16, tag="wqk")
            nc.sync.dma_start(
                wt[:],
                wqk[:, ft * P:(ft + 1) * P].rearrange("(kc p) f -> p kc f", p=P),
            )
            for t0 in (0, 288):
                psf = psA.tile([P, 512], f32, tag="mm", name="ps1a")
                ps = psf[:, :288]
                for kc in range(KC):
                    nc.tensor.matmul(
                        ps, wt[:, kc], xt_sb[:, kc, t0:t0 + 288],
                        start=(kc == 0), stop=(kc == KC - 1),
                    )
                nc.scalar.activation(
                    qkT_sb[:, ft, t0:t0 + 288], ps, Act.Identity,
                    bias=bqk_sb[:, ft:ft + 1],
                )

        # ---- phase 1b: v = x @ Wv  (token-major) ----
        for tt in range(NQT + 1):
            tsz = P if tt < NQT else W
            for n0 in (0, 512):
                psf = psA.tile([P, 512], f32, tag="mm", name="ps1b")
                ps = psf[:tsz]
                for kc in range(KC):
                    nc.tensor.matmul(
                        ps, xt_sb[:, kc, tt * P:tt * P + tsz],
                        wv_sb[:, kc, n0:n0 + 512],
                        start=(kc == 0), stop=(kc == KC - 1),
                    )
                nc.vector.tensor_add(
                    v_sb[:tsz, tt, n0:n0 + 512], ps, bvr_sb[:tsz, n0:n0 + 512]
                )

        # ---- phase 2: banded attention per (head pair, query tile) ----
        for hp in range(H // 2):
            for t in range(NQT):
                psS = psSp.tile([P, 2 * KW], f32, tag="psS")
                PT = at.tile([P, 2, 2 * P], bf16, tag="PT")
                for s in (0, 1):
                    r0 = 64 * s
                    Ssl = psS[:, s * KW:(s + 1) * KW]
                    nc.tensor.matmul(
                        Ssl,
                        qkT_sb[r0:r0 + 64, hp, W + t * P:W + (t + 1) * P],
                        qkT_sb[r0:r0 + 64, 8 + hp, t * P:t * P + KW],
                        start=True, stop=True,
                    )
                    Sm = at.tile([P, KW], f32, tag="Sm")
                    nc.vector.tensor_add(Sm[:], Ssl, mask_sb[:, t])
                    Pb = at.tile([P, KW], bf16, tag="Pb")
                    nc.scalar.activation(Pb[:], Sm[:], Act.Exp, scale=0.125)
                    rs = at.tile([P, 1], f32, tag="rs")
                    nc.vector.reduce_sum(rs[:], Pb[:], axis=X)
                    rr = at.tile([P, 1], f32, tag="rr")
                    nc.vector.reciprocal(rr[:], rs[:])
                    Pn = at.tile([P, KW], bf16, tag="Pn")
                    nc.vector.tensor_scalar_mul(Pn[:], Pb[:], rr[:])
                    psT = psTp.tile([P, 2 * P], bf16, tag="psT")
                    nc.tensor.transpose(psT[:, 0:P], Pn[:, 0:P], ident[:])
                    nc.tensor.transpose(psT[0:64, P:2 * P], Pn[:, P:KW], ident[:])
                    nc.vector.tensor_copy(PT[:, s, 0:P], psT[:, 0:P])
                    nc.vector.tensor_copy(PT[0:64, s, P:2 * P], psT[0:64, P:2 * P])
                psY = psYp.tile([P, P], f32, tag="psY")
                for s in (0, 1):
                    h = 2 * hp + s
                    ys = psY[64 * s:64 * s + 64, :]
                    nc.tensor.matmul(
                        ys, v_sb[:, t, h * 64:(h + 1) * 64], PT[:, s, 0:P],
                        start=True, stop=False, tile_position=(0, 64 * s),
                    )
                    nc.tensor.matmul(
                        ys, v_sb[0:64, t + 1, h * 64:(h + 1) * 64],
                        PT[0:64, s, P:2 * P],
                        start=False, stop=True, tile_position=(0, 64 * s),
                    )
                nc.vector.tensor_copy(yT_sb[:, hp, t * P:(t + 1) * P], psY[:])

        # ---- phase 3: out = y @ Wproj + b ----
        for tt in range(NQT):
            for n0 in (0, 512):
                ps = psA.tile([P, 512], f32, tag="mm", name="ps3")
                for kc in range(KC):
                    nc.tensor.matmul(
                        ps, yT_sb[:, kc, tt * P:(tt + 1) * P],
                        wp_sb[:, kc, n0:n0 + 512],
                        start=(kc == 0), stop=(kc == KC - 1),
                    )
                osb = ot.tile([P, 512], f32, tag="osb")
                nc.vector.tensor_add(osb[:], ps, bpr_sb[:, n0:n0 + 512])
                nc.sync.dma_start(out[tt * P:(tt + 1) * P, n0:n0 + 512], osb[:])


def _build():
    nc = bacc.Bacc(
        "TRN2", target_bir_lowering=False, debug=False,
        enable_asserts=True, num_devices=N_CORES,
    )
    xt = nc.dram_tensor("xt", [C, HT], bf16, kind="ExternalInput").ap()
    wqk = nc.dram_tensor("wqk", [C, 2 * C], bf16, kind="ExternalInput").ap()
    wv = nc.dram_tensor("wv", [C, C], bf16, kind="ExternalInput").ap()
    wp = nc.dram_tensor("wp", [C, C], bf16, kind="ExternalInput").ap()
    bqk = nc.dram_tensor("bqk", [2 * C], f32, kind="ExternalInput").ap()
    bvr = nc.dram_tensor("bvr", [P, C], f32, kind="ExternalInput").ap()
    bpr = nc.dram_tensor("bpr", [P, C], f32, kind="ExternalInput").ap()
    mask = nc.dram_tensor("mask", [NQT, P, KW], f32, kind="ExternalInput").ap()
    out = nc.dram_tensor("out", [QL, C], f32, kind="ExternalOutput").ap()
    with tile.TileContext(nc) as tc:
        _emit(tc, xt, wqk, wv, wp, bqk, bvr, bpr, mask, out)
    nc.compile()
    return nc


def _get_module():
    if "nc" not in _CACHE:
        _CACHE["nc"] = _build()
    return _CACHE["nc"]


def _band_mask(q_block: int) -> np.ndarray:
    i = np.arange(P)[:, None]
    j = np.arange(KW)[None, :]
    vis = (j >= i) & (j <= i + W)
    m = np.where(vis, 0.0, NEG).astype(np.float32)
    masks = np.stack([m] * NQT)
    if q_block == 0:
        # halo tokens (local key index < 64) are zero padding, not real keys
        masks[0] = np.where(vis & (j >= W), 0.0, NEG).astype(np.float32)
    return masks


def kernel(x, Wqkv, bqkv, Wproj, bproj):
    x = np.asarray(x, dtype=np.float32)
    Wqkv = np.asarray(Wqkv, dtype=np.float32)
    bqkv = np.asarray(bqkv, dtype=np.float32)
    Wproj = np.asarray(Wproj, dtype=np.float32)
    bproj = np.asarray(bproj, dtype=np.float32)

    bf = ml_dtypes.bfloat16
    wqk_np = np.ascontiguousarray(Wqkv[:, :2 * C]).astype(bf)
    wv_np = np.ascontiguousarray(Wqkv[:, 2 * C:]).astype(bf)
    wp_np = Wproj.astype(bf)
    bqk_np = np.ascontiguousarray(bqkv[:2 * C])
    bvr_np = np.ascontiguousarray(np.broadcast_to(bqkv[2 * C:], (P, C)))
    bpr_np = np.ascontiguousarray(np.broadcast_to(bproj, (P, C)))

    in_maps = []
    for c in range(N_CORES):
        b, q = divmod(c, 4)
        lo = q * QL - W
        if lo < 0:
            chunk = np.concatenate(
                [np.zeros((W, C), np.float32), x[b, 0:q * QL + QL]], axis=0
            )
        else:
            chunk = x[b, lo:lo + HT]
        in_maps.append({
            "xt": np.ascontiguousarray(chunk.T).astype(bf),
            "wqk": wqk_np,
            "wv": wv_np,
            "wp": wp_np,
            "bqk": bqk_np,
            "bvr": bvr_np,
            "bpr": bpr_np,
            "mask": _band_mask(q),
        })

    nc = _get_module()
    _CACHE["last_in_maps"] = in_maps
    res = bass_utils.run_bass_kernel_spmd(nc, in_maps, core_ids=list(range(N_CORES)))

    out = np.empty((B, T, C), dtype=np.float32)
    for c in range(N_CORES):
        b, q = divmod(c, 4)
        out[b, q * QL:(q + 1) * QL] = res.results[c]["out"]
    return out
